# revision 9
# baseline (speedup 1.0000x reference)
"""Multi-head attention with RoPE (B=2, S=2048, H=16 heads, D=64) on 8 TRN2
NeuronCores, tensor-parallel over heads (2 heads/core); host sums the 8
rank-128 partial outputs.

v1 structure (vs v0 phased baseline):
  - Prologue: batch-0 QKV projections (tcn 0..3) with per-tcn fused RoPE
    (rot = partition-swap DMA, rot*sin on GpSimd, mul/add on DVE).
  - Attention runs in 512-wide q chunks; per k-block the two heads' K=64
    score matmuls are issued to PE row-groups 0/64 and execute
    concurrently, writing the two halves of one [128,1024] PSUM tile;
    one Exp (ScalarE) covers both heads; PV (K=128) accumulates per-head
    ctx[65,512] (ones column -> softmax denominator l).
  - ScalarE does exp only during attention; batch-1 QKV + RoPE and the
    output projection are drained into attention's PE slack as filler
    units via a token-bucket (attention is exp-bound on ScalarE).
  - Normalize per chunk: l rows -> [128,8] scatter DMA, DVE reciprocal,
    DRAM bounce broadcast, fused into ctx psum->SBUF multiply.
  - Tail: remaining projection units on a wide PSUM pool, copies split
    DVE/ScalarE, stores split across DMA queues.
"""
import numpy as np
import ml_dtypes

import concourse.bass as bass
import concourse.mybir as mybir
import concourse.tile as tile
from concourse import bacc
from concourse.bass_utils import run_bass_kernel_spmd

F32 = mybir.dt.float32
F16 = mybir.dt.float16

B, S, HID = 2, 2048, 1024
NH, HD = 16, 64
T = B * S                  # 4096 tokens
NCORES = 8
HPC = NH // NCORES         # 2 heads per core
DPC = HPC * HD             # 128 context dims per core
ROPE_BASE = 10000.0

_CACHE = {}
DEBUG_TAPS = False


def _build_program():
    nc = bacc.Bacc("TRN2", target_bir_lowering=False, debug=False)

    xT_d = nc.dram_tensor("xT16", [HID, T], F16, kind="ExternalInput")
    wq_d = nc.dram_tensor("wq", [128, HID], F16, kind="ExternalInput")
    wk_d = nc.dram_tensor("wk", [128, HID], F16, kind="ExternalInput")
    wv_d = nc.dram_tensor("wv", [128, HID], F16, kind="ExternalInput")
    wo_d = nc.dram_tensor("wo", [DPC, HID], F16, kind="ExternalInput")
    cos_d = nc.dram_tensor("cosf", [128, S], F16, kind="ExternalInput")
    sin_d = nc.dram_tensor("sins", [128, S], F16, kind="ExternalInput")
    out_d = nc.dram_tensor("out", [T, HID], F32, kind="ExternalOutput")
    rscr_d = nc.dram_tensor("rscr", [8, 1024], F32)  # 1/l rows bounce

    dbg = None
    if DEBUG_TAPS:
        dbg = {
            "qT": nc.dram_tensor("dbg_qT", [128, T], F32, kind="ExternalOutput"),
            "kT": nc.dram_tensor("dbg_kT", [128, T], F32, kind="ExternalOutput"),
            "v0": nc.dram_tensor("dbg_v0", [128, 65], F32, kind="ExternalOutput"),
            "sp": nc.dram_tensor("dbg_sp", [128, 1024], F32, kind="ExternalOutput"),
            "p": nc.dram_tensor("dbg_p", [128, 1024], F32, kind="ExternalOutput"),
            "ctxps": nc.dram_tensor("dbg_ctxps", [65, 512], F32, kind="ExternalOutput"),
            "ctx0": nc.dram_tensor("dbg_ctx0", [128, S], F32, kind="ExternalOutput"),
            "rscr": nc.dram_tensor("dbg_rscr", [8, 1024], F32, kind="ExternalOutput"),
        }

    with tile.TileContext(nc) as tc:
        _emit(nc, tc, xT_d, wq_d, wk_d, wv_d, wo_d, cos_d, sin_d, out_d,
              rscr_d, dbg=dbg)
    nc.compile()
    return nc


def _emit(nc, tc, xT_d, wq_d, wk_d, wv_d, wo_d, cos_d, sin_d, out_d, rscr_d, dbg=None):
    import contextlib
    ctx = contextlib.ExitStack()
    with ctx:
        singles = ctx.enter_context(tc.tile_pool(name="singles", bufs=1))
        xpool = ctx.enter_context(tc.tile_pool(name="xpool", bufs=10))
        ppool = ctx.enter_context(tc.tile_pool(name="ppool", bufs=4))
        rotp = ctx.enter_context(tc.tile_pool(name="rotp", bufs=2))
        lpool = ctx.enter_context(tc.tile_pool(name="lpool", bufs=4))
        bpool = ctx.enter_context(tc.tile_pool(name="bpool", bufs=2))
        opool = ctx.enter_context(tc.tile_pool(name="opool", bufs=10))

        # ---- persistent SBUF ----
        wq_sb = singles.tile([128, 8, DPC], F16)
        wk_sb = singles.tile([128, 8, DPC], F16)
        wv_sb = singles.tile([128, 8, DPC], F16)
        wo_sb = singles.tile([128, HID], F16)
        cos_sb = singles.tile([128, S], F16)
        sin_sb = singles.tile([128, S], F16)
        qT_sb = singles.tile([128, T], F16)
        kT_sb = singles.tile([128, T], F16)
        v_all = singles.tile([128, 64, 65], F16)  # slot = h*32 + b*16 + kb
        ctx0_sb = singles.tile([128, S], F16)     # normalized ctx^T for b=0
        ctx1_sb = singles.tile([128, S], F16)

        nc.sync.dma_start(out=wq_sb[:].rearrange("p a b -> p (a b)"), in_=wq_d[:])
        nc.scalar.dma_start(out=wk_sb[:].rearrange("p a b -> p (a b)"), in_=wk_d[:])
        nc.gpsimd.dma_start(out=wv_sb[:].rearrange("p a b -> p (a b)"), in_=wv_d[:])
        nc.gpsimd.dma_start(out=wo_sb[:], in_=wo_d[:])
        nc.gpsimd.dma_start(out=cos_sb[:], in_=cos_d[:])
        nc.gpsimd.dma_start(out=sin_sb[:], in_=sin_d[:])
        nc.vector.memset(v_all[:, :, 64:65], 1.0)
        # preload the exp table set while ScalarE is idle
        tblw = singles.tile([1, 8], F32)
        nc.vector.memset(tblw[:], 0.0)
        nc.scalar.activation(out=tblw[:], in_=tblw[:],
                             func=mybir.ActivationFunctionType.Exp)

        # ---- RoPE on a 512-token slice of qT/kT (in place) ----
        def rope_tcn(t_sb, tcn, dq):
            b = tcn // 4
            tsl = slice(tcn * 512, (tcn + 1) * 512)
            cs = slice(tcn * 512 - b * S, (tcn + 1) * 512 - b * S)
            rot = rotp.tile([128, 512], F16, tag="rot", name="rot")
            dq.dma_start(out=rot[0:32, :], in_=t_sb[32:64, tsl])
            dq.dma_start(out=rot[32:64, :], in_=t_sb[0:32, tsl])
            dq.dma_start(out=rot[64:96, :], in_=t_sb[96:128, tsl])
            dq.dma_start(out=rot[96:128, :], in_=t_sb[64:96, tsl])
            nc.gpsimd.tensor_mul(rot[:], rot[:], sin_sb[:, cs])
            nc.vector.tensor_mul(t_sb[:, tsl], t_sb[:, tsl], cos_sb[:, cs])
            nc.vector.tensor_add(t_sb[:, tsl], t_sb[:, tsl], rot[:])

        # strided v_all write: [128,128] psum -> v slots (blk, blk+32)
        def v_store(src, blk):
            dst0 = v_all[:, blk, 0:64]
            dst = bass.AP(tensor=dst0.tensor, offset=dst0.offset,
                          ap=[list(dst0.ap[0]), [32 * 65, 2], [1, 64]])
            nc.vector.tensor_copy(dst, src)

        # ---- prologue: QKV + RoPE for batch 0 (tcn 0..3) ----
        with tc.tile_pool(name="qkps", bufs=2, space="PSUM") as qkps, \
             tc.tile_pool(name="vps", bufs=1, space="PSUM") as vps:
            with nc.named_scope("pro"):
                for tcn in range(4):
                    tsl = slice(tcn * 512, (tcn + 1) * 512)
                    psq = qkps.tile([128, 512], F32, tag="qk", name="psq")
                    psk = qkps.tile([128, 512], F32, tag="qk", name="psk")
                    # one tile (= one PSUM bank) per v sub-block: a start=True
                    # matmul clears its whole bank, so accumulation groups
                    # must not share banks
                    pvs = [vps.tile([128, DPC], F32, tag=f"pv{i}", name=f"pv{i}")
                           for i in range(4)]
                    for kc in range(8):
                        xt = xpool.tile([128, 512], F16, tag="x", name="xt")
                        dma_eng = nc.sync if kc % 2 == 0 else nc.scalar
                        dma_eng.dma_start(
                            out=xt[:],
                            in_=xT_d[kc * 128:(kc + 1) * 128, tsl])
                        st, sp = kc == 0, kc == 7
                        nc.tensor.matmul(psq[:], wq_sb[:, kc, :], xt[:], start=st, stop=sp)
                        nc.tensor.matmul(psk[:], wk_sb[:, kc, :], xt[:], start=st, stop=sp)
                        for sub in range(4):
                            nc.tensor.matmul(
                                pvs[sub][:],
                                xt[:, sub * 128:(sub + 1) * 128],
                                wv_sb[:, kc, :],
                                start=st, stop=sp)
                    nc.scalar.activation(out=qT_sb[:, tsl], in_=psq[:],
                                         func=mybir.ActivationFunctionType.Copy)
                    nc.vector.tensor_copy(kT_sb[:, tsl], psk[:])
                    for sub in range(4):
                        v_store(pvs[sub][:], tcn * 4 + sub)
                    rope_tcn(qT_sb, tcn, nc.sync)
                    rope_tcn(kT_sb, tcn, nc.sync)

        # ---- filler units for attention PE slack ----
        # each: (pe_ns_estimate, fn). Drained FIFO via token bucket.
        fillers = []
        fstate = {}
        cur_fill = [None]  # active PSUM pool for filler/proj matmuls

        def qk_unit(tcn, kc):
            def fn():
                tsl = slice(tcn * 512, (tcn + 1) * 512)
                if kc == 0:
                    fstate["qk"] = cur_fill[0].tile(
                        [128, 1024], F32, tag="fill", name="fqk")
                fqk = fstate["qk"]
                xt = xpool.tile([128, 512], F16, tag="x", name="xtf")
                dq = nc.sync if kc % 2 == 0 else nc.gpsimd
                dq.dma_start(out=xt[:], in_=xT_d[kc * 128:(kc + 1) * 128, tsl])
                st, sp = kc == 0, kc == 7
                nc.tensor.matmul(fqk[:, 0:512], wq_sb[:, kc, :], xt[:],
                                 start=st, stop=sp)
                nc.tensor.matmul(fqk[:, 512:1024], wk_sb[:, kc, :], xt[:],
                                 start=st, stop=sp)
            return (520, fn)

        def qk_copy(tcn):
            def fn():
                tsl = slice(tcn * 512, (tcn + 1) * 512)
                fqk = fstate.pop("qk")
                nc.vector.tensor_copy(qT_sb[:, tsl], fqk[:, 0:512])
                nc.vector.tensor_copy(kT_sb[:, tsl], fqk[:, 512:1024])
            return (80, fn)

        def rope_unit(which, tcn):
            def fn():
                rope_tcn(qT_sb if which == "q" else kT_sb, tcn, nc.gpsimd)
            return (50, fn)

        # v fillers are sub-major: one accumulation group (= one PSUM bank)
        # live at a time, copy-out between subs; xt tiles for the tcn are
        # stashed and reused across the 4 subs
        def v_unit(tcn, sub, kcg):
            def fn():
                tsl = slice(tcn * 512, (tcn + 1) * 512)
                if sub == 0:
                    for kc in (2 * kcg, 2 * kcg + 1):
                        xt = xpool.tile([128, 512], F16, tag="x", name="xtv")
                        dq = nc.sync if kc % 2 == 0 else nc.gpsimd
                        dq.dma_start(out=xt[:],
                                     in_=xT_d[kc * 128:(kc + 1) * 128, tsl])
                        fstate["vx", kc] = xt
                if kcg == 0:
                    fstate["v"] = cur_fill[0].tile(
                        [128, DPC], F32, tag="fill", name="fv")
                fv = fstate["v"]
                for kc in (2 * kcg, 2 * kcg + 1):
                    nc.tensor.matmul(
                        fv[:],
                        fstate["vx", kc][:, sub * 128:(sub + 1) * 128],
                        wv_sb[:, kc, :],
                        start=kc == 0, stop=kc == 7)
            return (260, fn)

        def v_copy(tcn, sub):
            def fn():
                fv = fstate.pop("v")
                v_store(fv[:], tcn * 4 + sub)
                if sub == 3:
                    for kc in range(8):
                        del fstate["vx", kc]
            return (80, fn)

        for tcn in range(4, 8):
            for kc in range(8):
                fillers.append(qk_unit(tcn, kc))
            fillers.append(qk_copy(tcn))
            fillers.append(rope_unit("q", tcn))
            fillers.append(rope_unit("k", tcn))
        for tcn in range(4, 8):
            for sub in range(4):
                for kcg in range(4):
                    fillers.append(v_unit(tcn, sub, kcg))
                fillers.append(v_copy(tcn, sub))

        # ---- output projection units ----
        projq = []

        def emit_proj_unit(pool, copy_eng=None, dma_eng=None, tag="fill"):
            bb, qb, oc = projq.pop(0)
            src = ctx0_sb if bb == 0 else ctx1_sb
            qsl = slice(qb * 128, (qb + 1) * 128)
            osl = slice(oc * 512, (oc + 1) * 512)
            ops = pool.tile([128, 512], F32, tag=tag, name="ops")
            nc.tensor.matmul(ops[:], src[:, qsl], wo_sb[:, osl],
                             start=True, stop=True)
            ot = opool.tile([128, 512], F32, tag="ot", name="ot")
            if copy_eng is nc.scalar:
                nc.scalar.activation(out=ot[:], in_=ops[:],
                                     func=mybir.ActivationFunctionType.Copy)
            else:
                nc.vector.tensor_copy(ot[:], ops[:])
            (dma_eng or nc.sync).dma_start(
                out=out_d[bb * S + qb * 128:bb * S + (qb + 1) * 128, osl],
                in_=ot[:])

        def proj_unit():
            def fn():
                if projq:
                    emit_proj_unit(cur_fill[0], dma_eng=nc.gpsimd)
            return (350, fn)

        # ---- attention: per batch, 512-wide q chunks, heads packed ----
        with tc.tile_pool(name="aps", bufs=2, space="PSUM") as aps, \
             tc.tile_pool(name="cps", bufs=1, space="PSUM") as cps:
            for b in range(B):
                fps = tc.tile_pool(
                    name=f"fps{b}", bufs=1 if b == 0 else 2, space="PSUM")
                fpool = fps.__enter__()
                cur_fill[0] = fpool
                ctx_sb = ctx0_sb if b == 0 else ctx1_sb
                with nc.named_scope(f"attn{b}"):
                    for qc in range(4):
                        q0 = b * S + qc * 512
                        qsl = slice(q0, q0 + 512)
                        csl = slice(qc * 512, (qc + 1) * 512)
                        ctxh = [cps.tile([65, 512], F32, tag=f"ctx{h}",
                                         name=f"ctx{h}")
                                for h in range(2)]
                        pring = {}
                        budget = 0.0
                        for kb in range(18):
                            if kb < 16:
                                k0 = b * S + kb * 128
                                ksl = slice(k0, k0 + 128)
                                sp_t = aps.tile([128, 1024], F32, tag="sp",
                                                name="sp_t")
                                nc.tensor.matmul(
                                    sp_t[:, 0:512],
                                    kT_sb[0:64, ksl], qT_sb[0:64, qsl],
                                    start=True, stop=True)
                                nc.tensor.matmul(
                                    sp_t[:, 512:1024],
                                    kT_sb[64:128, ksl], qT_sb[64:128, qsl],
                                    start=True, stop=True)
                                p_t = ppool.tile([128, 1024], F16, tag="p",
                                                 name="p_t")
                                nc.scalar.activation(
                                    out=p_t[:], in_=sp_t[:],
                                    func=mybir.ActivationFunctionType.Exp)
                                pring[kb] = p_t
                                if dbg is not None and b == 0 and qc == 0 and kb == 0:
                                    sp_sb = opool.tile([128, 1024], F32, tag="dbgs", name="dbgs")
                                    nc.vector.tensor_copy(sp_sb[:], sp_t[:])
                                    nc.sync.dma_start(out=dbg["sp"][:], in_=sp_sb[:])
                                    nc.gpsimd.dma_start(out=dbg["p"][:], in_=p_t[:])
                            if kb >= 2:
                                kv = kb - 2
                                p_t = pring.pop(kv)
                                st, sp = kv == 0, kv == 15
                                for h in range(2):
                                    sl_ = h * 32 + b * 16 + kv
                                    nc.tensor.matmul(
                                        ctxh[h][:, :], v_all[:, sl_, 0:65],
                                        p_t[:, h * 512:(h + 1) * 512],
                                        start=st, stop=sp)
                                budget = min(budget + 450.0, 1400.0)
                                while fillers and budget >= fillers[0][0]:
                                    cost, fn = fillers.pop(0)
                                    budget -= cost
                                    fn()
                        if dbg is not None and b == 0 and qc == 0:
                            cx_sb = opool.tile([65, 512], F32, tag="dbgc", name="dbgc")
                            nc.vector.tensor_copy(cx_sb[:], ctxh[0][:])
                            nc.sync.dma_start(out=dbg["ctxps"][:], in_=cx_sb[:])
                        # ---- normalize chunk: ctx/l ----
                        idx = b * 4 + qc
                        last = idx == 7
                        nq = nc.scalar if last else nc.sync
                        cuh = [lpool.tile([65, 512], F32, tag=f"cu{h}",
                                          name=f"cu{h}") for h in range(2)]
                        lcol = lpool.tile([128, 8], F32, tag="lcol",
                                          name="lcol")
                        for h in range(2):
                            nc.vector.tensor_copy(cuh[h][:], ctxh[h][:])
                            l0 = cuh[h][64:65, :]
                            nq.dma_start(
                                out=lcol[:, h * 4:(h + 1) * 4],
                                in_=bass.AP(tensor=l0.tensor, offset=l0.offset,
                                            ap=[list(l0.ap[0]), [4, 128], [1, 4]]))
                        nc.vector.reciprocal(lcol[:], lcol[:])
                        r0 = rscr_d[idx, :]
                        nq.dma_start(
                            out=bass.AP(tensor=r0.tensor, offset=r0.offset,
                                        ap=[[4, 128], [512, 2], [1, 4]]),
                            in_=lcol[:].rearrange("p (c j) -> p c j", c=2))
                        bct = bpool.tile([64, 1024], F32, tag="bct", name="bct")
                        nq.dma_start(
                            out=bct[:],
                            in_=bass.AP(tensor=r0.tensor, offset=r0.offset,
                                        ap=[[0, 64], [1, 1024]]))
                        for h in range(2):
                            rb = h * 64
                            nc.vector.tensor_mul(
                                ctx_sb[rb:rb + 64, csl], cuh[h][0:64, :],
                                bct[:, h * 512:(h + 1) * 512])
                        for qb in range(qc * 4, (qc + 1) * 4):
                            for oc in range(2):
                                projq.append((b, qb, oc))
                # drain remaining qkv/rope fillers at the phase boundary,
                # then refill with projection units for phase B
                if b == 0:
                    while fillers:
                        fillers.pop(0)[1]()
                    fillers = [proj_unit() for _ in range(64)]
                fps.__exit__(None, None, None)
                cur_fill[0] = None

        if dbg is not None:
            nc.gpsimd.dma_start(out=dbg["qT"][:], in_=qT_sb[:])
            nc.gpsimd.dma_start(out=dbg["kT"][:], in_=kT_sb[:])
            nc.gpsimd.dma_start(out=dbg["v0"][:], in_=v_all[:, 0, :])
            nc.gpsimd.dma_start(out=dbg["ctx0"][:], in_=ctx0_sb[:])
            nc.sync.dma_start(out=dbg["rscr"][:], in_=rscr_d[:])

        with tc.tile_pool(name="tps", bufs=6, space="PSUM") as tps:
            with nc.named_scope("projtail"):
                i = 0
                while projq:
                    emit_proj_unit(
                        tps,
                        copy_eng=nc.vector if (i // 2) % 2 == 0 else nc.scalar,
                        dma_eng=(nc.sync, nc.scalar)[i % 2],
                        tag="tp")
                    i += 1


def _swz(w):
    # [1024, 128] -> [128, 1024]: SBUF layout [p, kc*128+d] = w[kc*128+p, d]
    return np.ascontiguousarray(
        w.reshape(8, 128, 128).transpose(1, 0, 2).reshape(128, 1024))


def _prep_inputs(x, Wq, Wk, Wv, Wo):
    x2 = np.asarray(x, dtype=np.float32).reshape(T, HID)
    xT16 = np.ascontiguousarray(x2.T).astype(np.float16)

    half = HD // 2
    inv_freq = (1.0 / (ROPE_BASE ** (np.arange(half, dtype=np.float64) * 2.0 / HD)))
    ang = np.arange(S, dtype=np.float64)[None, :] * inv_freq[:, None]  # [32, S]
    cosf = np.tile(np.cos(ang), (4, 1)).astype(np.float16)
    sgn = np.repeat([-1.0, 1.0, -1.0, 1.0], 32)[:, None]
    sins = (np.tile(np.sin(ang), (4, 1)) * sgn).astype(np.float16)

    scale = np.float32(1.0 / np.sqrt(HD))
    in_maps = []
    for c in range(NCORES):
        rows = slice(c * DPC, (c + 1) * DPC)
        in_maps.append({
            "xT16": xT16,
            "wq": _swz((Wq[rows, :] * scale).T.astype(np.float16)),
            "wk": _swz(Wk[rows, :].T.astype(np.float16)),
            "wv": _swz(Wv[rows, :].T.astype(np.float16)),
            "wo": np.ascontiguousarray(Wo[:, rows].T).astype(np.float16),
            "cosf": cosf,
            "sins": sins,
        })
    return in_maps


def _run(in_maps, trace=False):
    if "nc" not in _CACHE:
        _CACHE["nc"] = _build_program()
    nc = _CACHE["nc"]
    res = run_bass_kernel_spmd(nc, in_maps, core_ids=list(range(NCORES)),
                               trace=trace)
    acc = res.results[0]["out"].astype(np.float32).copy()
    for c in range(1, NCORES):
        acc += res.results[c]["out"]
    return acc.reshape(B, S, HID), res


def kernel(x, Wq, Wk, Wv, Wo):
    in_maps = _prep_inputs(np.asarray(x), np.asarray(Wq), np.asarray(Wk),
                           np.asarray(Wv), np.asarray(Wo))
    out, _ = _run(in_maps, trace=False)
    return out


def run_profiled(x, Wq, Wk, Wv, Wo):
    in_maps = _prep_inputs(np.asarray(x), np.asarray(Wq), np.asarray(Wk),
                           np.asarray(Wv), np.asarray(Wo))
    return _run(in_maps, trace=True)
